# revision 1
# baseline (speedup 1.0000x reference)
"""Trainium2 Bass kernel for nn_DynaResidualBlockC (hyper-network dynamic
residual block).

Strategy (8 NeuronCores, data-parallel over batch):
  * Each core owns 2 of the 16 samples (samples 2c, 2c+1) and the full
    spatial extent for them.
  * The 51 MB hypernet weight Wk is sharded row-wise 8 ways (3200 rows per
    core after padding 24832 -> 25600).  Each core computes its shard of
    ks = lat @ Wk.T + bk for ALL 16 samples, then one AllToAll hands every
    core exactly its own 2 samples' complete kernel/bias vector.
  * Host-side preprocessing (pure marshalling): Wk rows are pre-scaled by
    the reference's 1/sqrt(fh) / 1/sqrt(fout) constants and permuted so
    that each per-sample conv kernel arrives in transposed ([in,out])
    layout, ready to be DMA'd straight into block-diagonal lhsT tiles.
    x / lat / Wk are pre-cast to bf16 (matmul dtype; fp32 PSUM accumulate).
  * Main loop: both samples are packed on the 128 SBUF partitions
    (64 channels each).  Per spatial tile of 2048 columns:
        psum_in  = W_in.T @ x2                      (PE, bf16)
        w1c/w1s  = sin(psum_in + b_in (+pi/2))      (ACT, fused bias)
        psum_mid = W_mid_c.T @ w1c + W_mid_s.T @ w1s
        w2c/w2s  = sin(psum_mid + b_mid (+pi/2))
        psum_out = W_out_c.T @ w2c + W_out_s.T @ w2s + W_short.T @ x2
        y        = psum_out + (b_out + b_short)     (DVE, fp32 out)
"""
import ml_dtypes
import numpy as np

import concourse.bass as bass
import concourse.bacc as bacc
import concourse.mybir as mybir
from concourse import tile
from concourse.bass_utils import run_bass_kernel_spmd

# ---------------------------------------------------------------- constants
B, FIN, FOUT, FH, H2 = 16, 64, 64, 128, 64
LAT = 512
HH = WW = 192
SP = HH * WW                      # 36864 spatial positions
KTOT, KPAD, SHARD = 24832, 25600, 3200
NCORES = 8
S_TILE = 512                      # spatial columns per main-loop tile
NT = SP // S_TILE                 # 72
NCH = S_TILE // 512               # 512-column matmul chunks per tile
PI_2 = float(np.pi / 2)

F32 = mybir.dt.float32
BF16 = mybir.dt.bfloat16
NP_BF16 = ml_dtypes.bfloat16


def _build_perm_scale():
    """orig-row index for each new (device-layout) Wk row, plus row scales."""
    orig = np.full(KPAD, -1, np.int64)
    j = np.arange(4096)
    orig[j] = (j % 64) * 64 + (j // 64)                        # k_in.T
    orig[4096 + j] = 4096 + (j % 64) * 128 + (j // 64)         # k_mid.T rows 0-63
    orig[8192 + j] = 4096 + (j % 64) * 128 + 64 + (j // 64)    # k_mid.T rows 64-127
    orig[12288 + j] = 12288 + (j % 64) * 128 + (j // 64)       # k_out.T rows 0-63
    orig[16384 + j] = 12288 + (j % 64) * 128 + 64 + (j // 64)  # k_out.T rows 64-127
    orig[20480 + j] = 20480 + (j % 64) * 64 + (j // 64)        # k_short.T
    orig[24576:24832] = np.arange(24576, 24832)                # biases, unpermuted
    s = np.ones(KTOT, np.float32)
    s[:12288] = 1.0 / np.sqrt(128.0)
    s[12288:24576] = 1.0 / 8.0
    return orig, s


def _build_nc():
    nc = bacc.Bacc(
        "TRN2",
        target_bir_lowering=False,
        debug=False,
        num_devices=NCORES,
    )
    x_d = nc.dram_tensor("x", [128, SP], BF16, kind="ExternalInput")
    latT_d = nc.dram_tensor("latT", [LAT, B], BF16, kind="ExternalInput")
    wkT_d = nc.dram_tensor("wkT", [LAT, SHARD], BF16, kind="ExternalInput")
    bk_d = nc.dram_tensor("bk", [1, SHARD], BF16, kind="ExternalInput")
    ones_d = nc.dram_tensor("onesr", [1, B], BF16, kind="ExternalInput")
    zeros_d = nc.dram_tensor("zeros", [64, 64], BF16, kind="ExternalInput")
    y_d = nc.dram_tensor("y", [128, SP], F32, kind="ExternalOutput")

    with tile.TileContext(nc) as tc:
        with (
            tc.tile_pool(name="wkt", bufs=1) as wkt_pool,
            tc.tile_pool(name="const", bufs=1) as const_pool,
            tc.tile_pool(name="wts", bufs=1) as w_pool,
            tc.tile_pool(name="dram", bufs=1, space="DRAM") as dram_pool,
            tc.tile_pool(name="psA", bufs=3, space=bass.MemorySpace.PSUM) as psA,
            tc.tile_pool(name="psB", bufs=3, space=bass.MemorySpace.PSUM) as psB,
            tc.tile_pool(name="psC", bufs=2, space=bass.MemorySpace.PSUM) as psC,
            tc.tile_pool(name="xin", bufs=8) as x_pool,
            tc.tile_pool(name="waves", bufs=4) as wave_pool,
            tc.tile_pool(name="outs", bufs=6) as out_pool,
        ):
            # ================= prologue: hypernet =================
            lat_tiles = []
            for q in range(4):
                lt = wkt_pool.tile([128, B], BF16, name=f"lat{q}", tag=f"lat{q}")
                nc.sync.dma_start(lt[:], latT_d[128 * q:128 * (q + 1), :])
                lat_tiles.append(lt)
            wkt_tiles = []
            for q in range(4):
                wt = wkt_pool.tile([128, SHARD], BF16, name=f"wkt{q}", tag=f"wkt{q}")
                nc.sync.dma_start(wt[:], wkT_d[128 * q:128 * (q + 1), :])
                wkt_tiles.append(wt)
            ones = const_pool.tile([1, B], BF16, name="ones")
            nc.sync.dma_start(ones[:], ones_d[:])
            bkrow = const_pool.tile([1, SHARD], BF16, name="bkrow")
            nc.sync.dma_start(bkrow[:], bk_d[:])
            ks_sb = const_pool.tile([B, SHARD], BF16, name="ks_sb")

            ci = 0
            n0 = 0
            while n0 < SHARD:
                nn = min(512, SHARD - n0)
                ps = psA.tile([B, 512], F32, name="hyps", tag="ps_in")
                for q in range(4):
                    nc.tensor.matmul(
                        ps[:, 0:nn],
                        lat_tiles[q][:],
                        wkt_tiles[q][:, n0:n0 + nn],
                        start=(q == 0),
                        stop=False,
                    )
                nc.tensor.matmul(
                    ps[:, 0:nn],
                    ones[:],
                    bkrow[:, n0:n0 + nn],
                    start=False,
                    stop=True,
                )
                nc.vector.tensor_copy(ks_sb[:, n0:n0 + nn], ps[:, 0:nn])
                n0 += nn
                ci += 1
            del ps

            # zero the off-diagonal blocks early (independent of the A2A)
            # and pre-trigger the trig ACT table load while PE does hypernet
            zscratch = const_pool.tile([1, B], F32, name="zscratch")
            nc.scalar.activation(zscratch[:], ones[:],
                                 mybir.ActivationFunctionType.Sin, bias=0.0)

            # weight lhsT tiles; zero the off-diagonal blocks early
            # (independent of the A2A, overlaps the hypernet)
            W_in = w_pool.tile([128, 128], BF16, name="W_in")
            W_mid_c = w_pool.tile([128, 128], BF16, name="W_mid_c")
            W_mid_s = w_pool.tile([128, 128], BF16, name="W_mid_s")
            W_out_c = w_pool.tile([128, 128], BF16, name="W_out_c")
            W_out_s = w_pool.tile([128, 128], BF16, name="W_out_s")
            W_short = w_pool.tile([128, 128], BF16, name="W_short")
            regions = [
                (W_in, 0), (W_mid_c, 4096), (W_mid_s, 8192),
                (W_out_c, 12288), (W_out_s, 16384), (W_short, 20480),
            ]
            for Wt, _ in regions:
                nc.gpsimd.dma_start(Wt[0:64, 64:128], zeros_d[:])
                nc.gpsimd.dma_start(Wt[64:128, 0:64], zeros_d[:])

            # ================= exchange: AllToAll =================
            cc_in = dram_pool.tile([B, SHARD], BF16, name="cc_in")
            cc_out = dram_pool.tile([B, SHARD], BF16, name="cc_out")
            n0 = 0
            while n0 < SHARD:
                nn = min(512, SHARD - n0)
                nc.gpsimd.dma_start(cc_in[:, n0:n0 + nn], ks_sb[:, n0:n0 + nn])
                n0 += nn
            nc.gpsimd.collective_compute(
                "AllToAll",
                mybir.AluOpType.bypass,
                replica_groups=[list(range(NCORES))],
                ins=[cc_in.opt()],
                outs=[cc_out.opt()],
            )

            # ============ per-sample weight/bias assembly ============
            for Wt, base in regions:
                for smp in (0, 1):
                    r = 0
                    while r < 64:
                        flat = base + r * 64
                        shard, col = divmod(flat, SHARD)
                        n = min(64 - r, (SHARD - col) // 64)
                        nc.sync.dma_start(
                            Wt[64 * smp + r:64 * smp + r + n,
                               64 * smp:64 * smp + 64],
                            cc_out[2 * shard + smp:2 * shard + smp + 1,
                                   col:col + 64 * n],
                        )
                        r += n

            bias_flat = const_pool.tile([2, 256], BF16, name="bias_flat")
            # biases live at flat [24576, 24832) -> shard 7, cols 2176:2432
            nc.gpsimd.dma_start(bias_flat[:], cc_out[14:16, 2176:2432])
            vin = const_pool.tile([128, 1], F32, name="vin")
            vmid = const_pool.tile([128, 1], F32, name="vmid")
            vout = const_pool.tile([128, 1], F32, name="vout")
            vsh = const_pool.tile([128, 1], F32, name="vsh")
            cvin = const_pool.tile([128, 1], F32, name="cvin")
            cvmid = const_pool.tile([128, 1], F32, name="cvmid")
            obias = const_pool.tile([128, 1], F32, name="obias")
            for smp in (0, 1):
                for q, dest in enumerate([vin, vmid, vout, vsh]):
                    # gpsimd DMA casts bf16 -> fp32 on the fly
                    nc.gpsimd.dma_start(
                        dest[64 * smp:64 * smp + 64, 0:1],
                        bias_flat[smp:smp + 1, 64 * q:64 * q + 64],
                    )
            nc.vector.tensor_scalar_add(cvin[:], vin[:], PI_2)
            nc.vector.tensor_scalar_add(cvmid[:], vmid[:], PI_2)
            nc.vector.tensor_add(obias[:], vout[:], vsh[:])

            # ================= main loop (2-deep software pipeline) ====
            # ACT stream alternates w1(t), w2(t-1); PE stream interleaves
            # in(t), out(t-2), mid(t-1) so neither engine waits on the
            # other's same-tile chain.
            SIN = mybir.ActivationFunctionType.Sin
            xts, w1s_, w2s_ = {}, {}, {}
            ps_ins, ps_mids = {}, {}
            for t in range(NT + 2):
                if t < NT:
                    c0 = t * S_TILE
                    xt = x_pool.tile([128, S_TILE], BF16, name="xt", tag="xt")
                    nc.sync.dma_start(xt[:], x_d[:, c0:c0 + S_TILE])
                    xts[t] = xt
                    ps_in = psA.tile([128, S_TILE], F32, name="ps_in",
                                     tag="ps_in")
                    for ch in range(NCH):
                        sl = np.s_[:, ch * 512:(ch + 1) * 512]
                        nc.tensor.matmul(ps_in[sl], W_in[:], xt[sl],
                                         start=True, stop=True)
                    ps_ins[t] = ps_in

                if t >= 2:
                    # out-stage for tile t-2
                    u = t - 2
                    w2c, w2s = w2s_.pop(u)
                    xt_u = xts.pop(u)
                    ps_out = psC.tile([128, S_TILE], F32, name="ps_out",
                                      tag="ps_out")
                    for ch in range(NCH):
                        sl = np.s_[:, ch * 512:(ch + 1) * 512]
                        nc.tensor.matmul(ps_out[sl], W_out_c[:], w2c[sl],
                                         start=True, stop=False)
                        nc.tensor.matmul(ps_out[sl], W_out_s[:], w2s[sl],
                                         start=False, stop=False)
                        nc.tensor.matmul(ps_out[sl], W_short[:], xt_u[sl],
                                         start=False, stop=True)
                    ot = out_pool.tile([128, S_TILE], F32, name="ot", tag="ot")
                    nc.vector.tensor_scalar_add(ot[:], ps_out[:], obias[:, 0:1])
                    nc.sync.dma_start(y_d[:, u * S_TILE:(u + 1) * S_TILE], ot[:])

                if 1 <= t <= NT:
                    # mid-stage for tile t-1
                    v = t - 1
                    w1c, w1s = w1s_.pop(v)
                    ps_mid = psB.tile([128, S_TILE], F32, name="ps_mid",
                                      tag="ps_mid")
                    for ch in range(NCH):
                        sl = np.s_[:, ch * 512:(ch + 1) * 512]
                        nc.tensor.matmul(ps_mid[sl], W_mid_c[:], w1c[sl],
                                         start=True, stop=False)
                        nc.tensor.matmul(ps_mid[sl], W_mid_s[:], w1s[sl],
                                         start=False, stop=True)
                    ps_mids[v] = ps_mid

                if t < NT:
                    # wave-1 ACTs for tile t
                    w1c = wave_pool.tile([128, S_TILE], BF16, name="w1c",
                                         tag="w1c")
                    w1s = wave_pool.tile([128, S_TILE], BF16, name="w1s",
                                         tag="w1s")
                    nc.scalar.activation(w1c[:], ps_ins[t][:], SIN,
                                         bias=cvin[:, 0:1])
                    nc.scalar.activation(w1s[:], ps_ins[t][:], SIN,
                                         bias=vin[:, 0:1])
                    ps_ins.pop(t)
                    w1s_[t] = (w1c, w1s)

                if 1 <= t <= NT:
                    # wave-2 ACTs for tile t-1
                    v = t - 1
                    w2c = wave_pool.tile([128, S_TILE], BF16, name="w2c",
                                         tag="w2c")
                    w2s = wave_pool.tile([128, S_TILE], BF16, name="w2s",
                                         tag="w2s")
                    nc.scalar.activation(w2c[:], ps_mids[v][:], SIN,
                                         bias=cvmid[:, 0:1])
                    nc.scalar.activation(w2s[:], ps_mids[v][:], SIN,
                                         bias=vmid[:, 0:1])
                    ps_mids.pop(v)
                    w2s_[v] = (w2c, w2s)

    nc.compile()
    return nc


_NC_CACHE = None


def _get_nc():
    global _NC_CACHE
    if _NC_CACHE is None:
        _NC_CACHE = _build_nc()
    return _NC_CACHE


def kernel(x, lat, Wk, bk, **run_kwargs):
    x = np.asarray(x, dtype=np.float32)
    lat = np.asarray(lat, dtype=np.float32)
    Wk = np.asarray(Wk, dtype=np.float32)
    bk = np.asarray(bk, dtype=np.float32)

    orig, s = _build_perm_scale()
    Wk_s = Wk * s[:, None]
    bk_s = bk * s
    Wk_new = np.zeros((KPAD, LAT), np.float32)
    bk_new = np.zeros(KPAD, np.float32)
    valid = orig >= 0
    Wk_new[valid] = Wk_s[orig[valid]]
    bk_new[valid] = bk_s[orig[valid]]
    latT_b = np.ascontiguousarray(lat.T.astype(NP_BF16))
    x_b = x.reshape(B, FIN * SP).astype(NP_BF16)

    in_maps = []
    for c in range(NCORES):
        in_maps.append({
            "x": np.ascontiguousarray(
                x_b[2 * c:2 * c + 2].reshape(128, SP)),
            "latT": latT_b,
            "wkT": np.ascontiguousarray(
                Wk_new[c * SHARD:(c + 1) * SHARD].T.astype(NP_BF16)),
            "bk": np.ascontiguousarray(
                bk_new[c * SHARD:(c + 1) * SHARD].reshape(1, SHARD).astype(NP_BF16)),
            "onesr": np.ones((1, B), NP_BF16),
            "zeros": np.zeros((64, 64), NP_BF16),
        })

    nc = _get_nc()
    res = run_bass_kernel_spmd(nc, in_maps, core_ids=list(range(NCORES)),
                               **run_kwargs)
    y = np.empty((B, FOUT, HH, WW), np.float32)
    for c in range(NCORES):
        y[2 * c:2 * c + 2] = res.results[c]["y"].reshape(2, FOUT, HH, WW)
    if run_kwargs:
        kernel.last_results = res
    return y



# revision 4
# speedup vs baseline: 1.2545x; 1.2545x over previous
"""Trainium2 Bass kernel for nn_DynaResidualBlockC (hyper-network dynamic
residual block).

Strategy (8 NeuronCores, data-parallel over batch):
  * Each core owns 2 of the 16 samples (samples 2c, 2c+1) and the full
    spatial extent for them.
  * The hypernet weight Wk is sharded row-wise 8 ways with an interleaved
    per-shard layout [biases | k_in | k_mid | k_out | k_short] so every
    shard holds a 1/8 slice of EVERY region; each region slice lands
    contiguously after the exchange, so every block-diagonal weight tile
    assembles with ONE flat DMA per (region, sample).  Each core computes
    its shard of ks = lat @ Wk.T + bk for ALL 16 samples; one AllToAll
    hands every core its own 2 samples' complete kernel/bias vector.
    (Multiple staged collectives were tried and are pathologically slow —
    ~45us each vs ~7us for a single one.)
  * Host-side preprocessing (pure marshalling): Wk rows are pre-scaled by
    the reference's 1/sqrt(fh) / 1/sqrt(fout) constants and permuted so
    each per-sample conv kernel arrives in transposed ([in,out]) layout.
    All inputs are pre-cast to bf16.
  * Prologue latency tricks: weight DMAs split across the sync+scalar
    queues in column halves so the first hypernet chunk starts ~5us in;
    a burst of tiny dummy matmuls warms the PE (HAM clock gate) before
    the real work; the ACT trig table loads at t~0.
  * Main loop, 24 macro-tiles of 1536 spatial columns (both samples packed
    on the 128 SBUF partitions).  PSUM: ps_in 3 banks + ps_mid 3 banks
    (single-buffered [128,1536] f32) + 2x [128,512] ps_out ping-pong = 8.
    Per macro-tile:
        ps_in  = W_in.T @ x2                       (PE, 3x512 chunks)
        w1c/w1s = sin(ps_in + b_in (+pi/2))        (ACT, N=1536 fused bias)
        ps_mid = W_mid_c.T @ w1c + W_mid_s.T @ w1s (PE)
        w2c/w2s = sin(ps_mid + b_mid (+pi/2))      (ACT)
        ps_out = W_out_c.T@w2c + W_out_s.T@w2s + W_short.T@x2  (512 chunks)
        y      = ps_out + (b_out + b_short)        (DVE, bf16 out)
    ACT is the bottleneck engine (~151us busy); the schedule keeps it
    saturated: per period ACT does w1c(t),w1s(t),w2c(t-1),w2s(t-1) while
    PE fits in(t+1), mid(t), out(t-1) in the gaps.  The 1536-wide ACT
    instructions amortize the 352-cycle per-instruction overhead that
    dominated at 512.
  * y is stored bf16 (halves store traffic); host casts back to fp32.
"""
import ml_dtypes
import numpy as np

import concourse.bass as bass
import concourse.bacc as bacc
import concourse.mybir as mybir
from concourse import tile
from concourse.bass_utils import run_bass_kernel_spmd

# ---------------------------------------------------------------- constants
B, FIN, FOUT, FH, H2 = 16, 64, 64, 128, 64
LAT = 512
HH = WW = 192
SP = HH * WW                      # 36864 spatial positions
KTOT = 24832
NCORES = 8
SHARD = KTOT // NCORES            # 3104 rows per core, exact
# per-shard column layout: [biases 32 | k_in 512 | k_mid 1024 | k_out 1024
#                           | k_short 512]
S_TILE = 1536                     # spatial columns per main-loop macro-tile
NT = SP // S_TILE                 # 24
NCH = S_TILE // 512               # 3 x 512-col matmul chunks per tile
WKH = 1536                        # wk DMA column-split point
PI_2 = float(np.pi / 2)

F32 = mybir.dt.float32
BF16 = mybir.dt.bfloat16
NP_BF16 = ml_dtypes.bfloat16


def _build_perm_scale():
    """orig-row index for each (shard, shard-col) position, plus row scales."""
    orig = np.zeros((NCORES, SHARD), np.int64)
    j8 = np.arange(8)
    t512 = np.arange(512)
    t1024 = np.arange(1024)
    for c in range(NCORES):
        orig[c, 0:8] = 24576 + 8 * c + j8            # b_in slice
        orig[c, 8:16] = 24640 + 8 * c + j8           # b_mid slice
        orig[c, 16:24] = 24704 + 8 * c + j8          # b_out slice
        orig[c, 24:32] = 24768 + 8 * c + j8          # b_short slice
        i, o = (512 * c + t512) // 64, (512 * c + t512) % 64
        orig[c, 32:544] = o * 64 + i                 # k_in.T slice
        i, o = (1024 * c + t1024) // 64, (1024 * c + t1024) % 64
        orig[c, 544:1568] = 4096 + o * 128 + i       # k_mid.T slice
        orig[c, 1568:2592] = 12288 + o * 128 + i     # k_out.T slice
        i, o = (512 * c + t512) // 64, (512 * c + t512) % 64
        orig[c, 2592:3104] = 20480 + o * 64 + i      # k_short.T slice
    s = np.ones(KTOT, np.float32)
    s[:12288] = 1.0 / np.sqrt(128.0)
    s[12288:24576] = 0.125
    return orig, s


def _build_nc():
    nc = bacc.Bacc(
        "TRN2",
        target_bir_lowering=False,
        debug=False,
        num_devices=NCORES,
    )
    x_d = nc.dram_tensor("x", [128, SP], BF16, kind="ExternalInput")
    latT_d = nc.dram_tensor("latT", [LAT, B], BF16, kind="ExternalInput")
    wkT_d = nc.dram_tensor("wkT", [LAT, SHARD], BF16, kind="ExternalInput")
    bk_d = nc.dram_tensor("bk", [1, SHARD], BF16, kind="ExternalInput")
    ones_d = nc.dram_tensor("onesr", [1, B], BF16, kind="ExternalInput")
    y_d = nc.dram_tensor("y", [128, SP], BF16, kind="ExternalOutput")

    SIN = mybir.ActivationFunctionType.Sin

    with tile.TileContext(nc) as tc:
        with (
            tc.tile_pool(name="wkt", bufs=1) as wkt_pool,
            tc.tile_pool(name="const", bufs=1) as const_pool,
            tc.tile_pool(name="wts", bufs=1) as w_pool,
            tc.tile_pool(name="dram", bufs=1, space="DRAM") as dram_pool,
            tc.tile_pool(name="psA", bufs=1, space=bass.MemorySpace.PSUM) as psA,
            tc.tile_pool(name="psB", bufs=1, space=bass.MemorySpace.PSUM) as psB,
            tc.tile_pool(name="psC", bufs=2, space=bass.MemorySpace.PSUM) as psC,
            tc.tile_pool(name="xin", bufs=5) as x_pool,
            tc.tile_pool(name="waves", bufs=2) as wave_pool,
            tc.tile_pool(name="outs", bufs=2) as out_pool,
        ):
            # ---------- sync queue: lat first, then wk column-halves ------
            lat_tiles = []
            for q in range(4):
                lt = wkt_pool.tile([128, B], BF16, name=f"lat{q}", tag=f"lat{q}")
                nc.sync.dma_start(lt[:], latT_d[128 * q:128 * (q + 1), :])
                lat_tiles.append(lt)
            ones = const_pool.tile([1, B], BF16, name="ones")
            nc.sync.dma_start(ones[:], ones_d[:])
            wk_tiles = []
            for q in range(4):
                wt = wkt_pool.tile([128, SHARD], BF16, name=f"wk{q}",
                                   tag=f"wk{q}")
                wk_tiles.append(wt)
            for q in (0, 1):
                nc.sync.dma_start(wk_tiles[q][:, 0:WKH],
                                  wkT_d[128 * q:128 * (q + 1), 0:WKH])
            for q in (0, 1):
                nc.sync.dma_start(wk_tiles[q][:, WKH:SHARD],
                                  wkT_d[128 * q:128 * (q + 1), WKH:SHARD])
            bkrow = const_pool.tile([1, SHARD], BF16, name="bkrow")
            nc.sync.dma_start(bkrow[:], bk_d[:])
            xts = {}
            for t in range(4):
                xt = x_pool.tile([128, S_TILE], BF16, name="xt", tag="xt")
                nc.sync.dma_start(xt[:], x_d[:, t * S_TILE:(t + 1) * S_TILE])
                xts[t] = xt

            # ---------- scalar queue: wk halves for q=2,3 + ACT trig-table
            # preload (ACT has no real work until the first w1)
            for q in (2, 3):
                nc.scalar.dma_start(wk_tiles[q][:, 0:WKH],
                                    wkT_d[128 * q:128 * (q + 1), 0:WKH])
            zscratch = const_pool.tile([128, 1], F32, name="zscratch")
            nc.scalar.activation(zscratch[:], lat_tiles[0][:, 0:1], SIN,
                                 bias=0.0)
            for q in (2, 3):
                nc.scalar.dma_start(wk_tiles[q][:, WKH:SHARD],
                                    wkT_d[128 * q:128 * (q + 1), WKH:SHARD])

            # ---------- PE warm-up: tiny dummy matmuls release the HAM
            # clock gate (~3.4us of sustained activity) before the real work
            for _ in range(36):
                dps = psA.tile([B, B], F32, name="warm", tag="ps_in")
                nc.tensor.matmul(dps[:], lat_tiles[0][:], lat_tiles[1][:],
                                 start=True, stop=True)

            # main-loop weight tiles; zero them early (gpsimd queue)
            W_in = w_pool.tile([128, 128], BF16, name="W_in")
            W_mid_c = w_pool.tile([128, 128], BF16, name="W_mid_c")
            W_mid_s = w_pool.tile([128, 128], BF16, name="W_mid_s")
            W_out_c = w_pool.tile([128, 128], BF16, name="W_out_c")
            W_out_s = w_pool.tile([128, 128], BF16, name="W_out_s")
            W_short = w_pool.tile([128, 128], BF16, name="W_short")
            for Wt in (W_in, W_mid_c, W_mid_s, W_out_c, W_out_s, W_short):
                nc.gpsimd.memset(Wt[:], 0.0)

            # ---------- hypernet: ks shard = lat @ Wk_c.T + bk ------------
            ks_sb = const_pool.tile([B, SHARD], BF16, name="ks_sb")
            n0 = 0
            while n0 < SHARD:
                nn = min(512, SHARD - n0)
                ps = psC.tile([B, nn], F32, name="hyps", tag="ps_out")
                for q in range(4):
                    nc.tensor.matmul(
                        ps[:],
                        lat_tiles[q][:],
                        wk_tiles[q][:, n0:n0 + nn],
                        start=(q == 0),
                        stop=False,
                    )
                nc.tensor.matmul(
                    ps[:],
                    ones[:],
                    bkrow[:, n0:n0 + nn],
                    start=False,
                    stop=True,
                )
                nc.vector.tensor_copy(ks_sb[:, n0:n0 + nn], ps[:])
                n0 += nn

            # ---------- exchange (single AllToAll) + assembly -------------
            # [16, 3104] sbuf rows (samples) == [8, 6208] dram rows (pairs)
            cc_in = dram_pool.tile([NCORES, 2 * SHARD], BF16, name="cc_in")
            cc_out = dram_pool.tile([NCORES, 2 * SHARD], BF16, name="cc_out")
            nc.gpsimd.dma_start(cc_in[:], ks_sb[:])
            nc.gpsimd.collective_compute(
                "AllToAll",
                mybir.AluOpType.bypass,
                replica_groups=[list(range(NCORES))],
                ins=[cc_in.opt()],
                outs=[cc_out.opt()],
            )

            # gpsimd: W_in + biases first (unblocks stage 1), then out-stage
            # weights.  sync: W_mid (needed one period later).
            bias_flat = const_pool.tile([8, 64], BF16, name="bias_flat")
            vin = const_pool.tile([128, 1], F32, name="vin")
            vmid = const_pool.tile([128, 1], F32, name="vmid")
            vout = const_pool.tile([128, 1], F32, name="vout")
            vsh = const_pool.tile([128, 1], F32, name="vsh")
            cvin = const_pool.tile([128, 1], F32, name="cvin")
            cvmid = const_pool.tile([128, 1], F32, name="cvmid")
            obias = const_pool.tile([128, 1], F32, name="obias")
            for smp in (0, 1):
                base = SHARD * smp
                nc.gpsimd.dma_start(
                    W_in[64 * smp:64 * smp + 64, 64 * smp:64 * smp + 64],
                    cc_out[:, base + 32:base + 544],
                )
                nc.gpsimd.dma_start(bias_flat[:, 32 * smp:32 * smp + 32],
                                    cc_out[:, base:base + 32])
                nc.sync.dma_start(
                    W_mid_c[64 * smp:64 * smp + 64, 64 * smp:64 * smp + 64],
                    cc_out[0:4, base + 544:base + 1568],
                )
                nc.sync.dma_start(
                    W_mid_s[64 * smp:64 * smp + 64, 64 * smp:64 * smp + 64],
                    cc_out[4:8, base + 544:base + 1568],
                )
            for smp in (0, 1):
                for q, dest in enumerate([vin, vmid, vout, vsh]):
                    # gpsimd DMA casts bf16 -> fp32 on the fly
                    nc.gpsimd.dma_start(
                        dest[64 * smp:64 * smp + 64, 0:1],
                        bias_flat[0:8, 32 * smp + 8 * q:32 * smp + 8 * q + 8],
                    )
            nc.vector.tensor_scalar_add(cvin[:], vin[:], PI_2)
            nc.vector.tensor_scalar_add(cvmid[:], vmid[:], PI_2)
            nc.vector.tensor_add(obias[:], vout[:], vsh[:])
            for smp in (0, 1):
                base = SHARD * smp
                nc.gpsimd.dma_start(
                    W_out_c[64 * smp:64 * smp + 64, 64 * smp:64 * smp + 64],
                    cc_out[0:4, base + 1568:base + 2592],
                )
                nc.gpsimd.dma_start(
                    W_out_s[64 * smp:64 * smp + 64, 64 * smp:64 * smp + 64],
                    cc_out[4:8, base + 1568:base + 2592],
                )
                nc.gpsimd.dma_start(
                    W_short[64 * smp:64 * smp + 64, 64 * smp:64 * smp + 64],
                    cc_out[:, base + 2592:base + 3104],
                )

            # ================= main loop =================
            # ACT order per period: w1c(t), w1s(t), w2c(t-1), w2s(t-1)
            # PE order:             in(t+1), mid(t), out(t-1)
            ps_ins, ps_mids, w1_, w2_ = {}, {}, {}, {}

            ps_in = psA.tile([128, S_TILE], F32, name="ps_in", tag="ps_in")
            for ch in range(NCH):
                sl = np.s_[:, ch * 512:(ch + 1) * 512]
                nc.tensor.matmul(ps_in[sl], W_in[:], xts[0][sl],
                                 start=True, stop=True)
            ps_ins[0] = ps_in

            for t in range(NT + 1):
                if t < NT:
                    # wave-1 ACTs for tile t
                    w1c = wave_pool.tile([128, S_TILE], BF16, name="w1c",
                                         tag="w1c")
                    w1s = wave_pool.tile([128, S_TILE], BF16, name="w1s",
                                         tag="w1s")
                    nc.scalar.activation(w1c[:], ps_ins[t][:], SIN,
                                         bias=cvin[:, 0:1])
                    nc.scalar.activation(w1s[:], ps_ins[t][:], SIN,
                                         bias=vin[:, 0:1])
                    ps_ins.pop(t)
                    w1_[t] = (w1c, w1s)

                if t + 1 < NT:
                    # in-stage for tile t+1 (reuses the single ps_in buffer
                    # as soon as w1s(t) has drained it)
                    ps_in = psA.tile([128, S_TILE], F32, name="ps_in",
                                     tag="ps_in")
                    for ch in range(NCH):
                        sl = np.s_[:, ch * 512:(ch + 1) * 512]
                        nc.tensor.matmul(ps_in[sl], W_in[:], xts[t + 1][sl],
                                         start=True, stop=True)
                    ps_ins[t + 1] = ps_in

                if t >= 1:
                    # wave-2 ACTs for tile t-1
                    v = t - 1
                    w2c = wave_pool.tile([128, S_TILE], BF16, name="w2c",
                                         tag="w2c")
                    w2s = wave_pool.tile([128, S_TILE], BF16, name="w2s",
                                         tag="w2s")
                    nc.scalar.activation(w2c[:], ps_mids[v][:], SIN,
                                         bias=cvmid[:, 0:1])
                    nc.scalar.activation(w2s[:], ps_mids[v][:], SIN,
                                         bias=vmid[:, 0:1])
                    ps_mids.pop(v)
                    w2_[v] = (w2c, w2s)

                if t < NT:
                    # mid-stage for tile t
                    w1c, w1s = w1_.pop(t)
                    ps_mid = psB.tile([128, S_TILE], F32, name="ps_mid",
                                      tag="ps_mid")
                    for ch in range(NCH):
                        sl = np.s_[:, ch * 512:(ch + 1) * 512]
                        nc.tensor.matmul(ps_mid[sl], W_mid_c[:], w1c[sl],
                                         start=True, stop=False)
                        nc.tensor.matmul(ps_mid[sl], W_mid_s[:], w1s[sl],
                                         start=False, stop=True)
                    ps_mids[t] = ps_mid

                if t >= 1:
                    # out-stage for tile t-1, 512-col chunks through the
                    # 2-bank ps_out ping-pong, drained by DVE (+bias, bf16)
                    v = t - 1
                    w2c, w2s = w2_.pop(v)
                    xt_v = xts.pop(v)
                    ot = out_pool.tile([128, S_TILE], BF16, name="ot", tag="ot")
                    for ch in range(NCH):
                        sl = np.s_[:, ch * 512:(ch + 1) * 512]
                        ps_out = psC.tile([128, 512], F32, name="ps_out",
                                          tag="ps_out")
                        nc.tensor.matmul(ps_out[:], W_out_c[:], w2c[sl],
                                         start=True, stop=False)
                        nc.tensor.matmul(ps_out[:], W_out_s[:], w2s[sl],
                                         start=False, stop=False)
                        nc.tensor.matmul(ps_out[:], W_short[:], xt_v[sl],
                                         start=False, stop=True)
                        nc.vector.tensor_scalar_add(ot[sl], ps_out[:],
                                                    obias[:, 0:1])
                    nc.gpsimd.dma_start(
                        y_d[:, v * S_TILE:(v + 1) * S_TILE], ot[:])

                if t + 4 < NT:
                    # x prefetch (sync queue), 5-buffer pool covers live
                    # range {t-1 .. t+3}
                    xt = x_pool.tile([128, S_TILE], BF16, name="xt", tag="xt")
                    nc.sync.dma_start(
                        xt[:], x_d[:, (t + 4) * S_TILE:(t + 5) * S_TILE])
                    xts[t + 4] = xt

    nc.compile()
    return nc


_NC_CACHE = None


def _get_nc():
    global _NC_CACHE
    if _NC_CACHE is None:
        _NC_CACHE = _build_nc()
    return _NC_CACHE


def kernel(x, lat, Wk, bk, **run_kwargs):
    x = np.asarray(x, dtype=np.float32)
    lat = np.asarray(lat, dtype=np.float32)
    Wk = np.asarray(Wk, dtype=np.float32)
    bk = np.asarray(bk, dtype=np.float32)

    orig, s = _build_perm_scale()
    Wk_s = Wk * s[:, None]
    bk_s = bk * s
    latT_b = np.ascontiguousarray(lat.T.astype(NP_BF16))
    x_b = x.reshape(B, FIN * SP).astype(NP_BF16)

    in_maps = []
    for c in range(NCORES):
        rows = orig[c]
        in_maps.append({
            "x": np.ascontiguousarray(
                x_b[2 * c:2 * c + 2].reshape(128, SP)),
            "latT": latT_b,
            "wkT": np.ascontiguousarray(Wk_s[rows].T.astype(NP_BF16)),
            "bk": np.ascontiguousarray(
                bk_s[rows].reshape(1, SHARD).astype(NP_BF16)),
            "onesr": np.ones((1, B), NP_BF16),
        })

    nc = _get_nc()
    res = run_bass_kernel_spmd(nc, in_maps, core_ids=list(range(NCORES)),
                               **run_kwargs)
    y = np.empty((B, FOUT, HH, WW), np.float32)
    for c in range(NCORES):
        y[2 * c:2 * c + 2] = res.results[c]["y"].astype(
            np.float32).reshape(2, FOUT, HH, WW)
    if run_kwargs:
        kernel.last_results = res
    return y
